# revision 81
# baseline (speedup 1.0000x reference)
"""Trainium2 Bass kernel for nn_AdaptiveResBlock (8-core data-parallel).

Reference computation (per batch element b, C=256 channels, T=8192 time):
  for i, dil in enumerate((1, 2, 4)):
      xt = lrelu(x)
      xP, xF = time-gather of xt at round(t -/+ d*dil), zero out-of-range
      xt = WC@xt + WP@xP + WF@xF + biases        (1x1 convs over channels)
      xt = lrelu(xt)
      xt = conv3(xt, WA) + bias
      x = xt + x

Structure used:
  * The time-gather commutes with the 1x1 convs:
    WP @ gather(xt) == gather(WP @ xt).
  * B-stage: u^T = [WP;WF] @ xt computed token-major on the PE
    (xt-stationary matmuls), ACT-copied as bf16 st tiles [128 tok, 512].
  * Channel-major windowed one-hot gather: offsets are bounded by
    16*dilation, so every output token whose P-source lies in token
    block j sits in the (128+pad)-token window [128j, 128j+128+pad)
    with pad = 16*dil (F: [128j-pad, 128j+128)).  One matmul per
    (dir, c-half, src block):
      psum[c, t] += st[j][:, c-slice]^T @ S_dir[j][src, t-window]
    lands the gathered conv result DIRECTLY channel-major in PSUM, on top
    of the WC matmuls accumulating in the same bank -- no PE transposes,
    no separate gather pass.  S is built on DVE via is_equal of
    preloaded window-relative indices (int8, -128 sentinel) against an
    iota column; out-of-range indices match no block => free zero mask.
  * Gather windows are dilation-aware: iteration i only spans
    128 + 16*dil source tokens per block, cutting one-hot matmul
    columns by ~14% versus a fixed 192-token window.
  * v = Prelu(psum) straight from PSUM (ACT); conv3 runs channel-major
    off v tiles with 1-column halos; residual update and the next
    iteration's lrelu are fused right behind each tile (DVE).
  * Only lrelu(x) is shipped (host-precomputed bf16): iteration-0's xt
    tiles come straight off DMA with no on-chip lrelu, and the residual
    base is recovered exactly via x = min(10*lrelu(x), lrelu(x)) fused
    into iteration-0's conv3 drain.  The residual accumulates in bf16
    (every matmul consumer is bf16 anyway); the final iteration drains
    through fp32 staging tiles for the output DMA.
  * All DMAs dispatch from the sync engine in need-time order (the
    DMA-semaphore flow control then paces transfers by priority);
    weights are stored in DRAM pre-transposed to the exact SBUF layout
    so every weight DMA is a contiguous per-partition row; iteration
    i+1's weights/rel prefetch mid-iteration i.

Sharded data-parallel over B=8 across the 8 NeuronCores; weights
replicated; per-core window-index tensors precomputed from d on host.
"""

import numpy as np
import ml_dtypes
from contextlib import ExitStack

import concourse.bass as bass
import concourse.tile as tile
from concourse import mybir, bacc
from concourse.bass_utils import run_bass_kernel_spmd

F32 = mybir.dt.float32
BF16 = mybir.dt.bfloat16
AF = mybir.ActivationFunctionType
OP = mybir.AluOpType

B, C, T_FULL = 8, 256, 8192
DILATIONS = (1, 2, 4)
PADS = tuple(16 * d for d in DILATIONS)   # max gather offset per iteration
NITER = len(DILATIONS)
SLOPE = 0.1
LAG = 8        # B-stage leads the consume stage by this many token blocks


def _pieces(tt, nR, T, pad):
    """Gather matmul pieces for 512-token tile tt at max offset `pad`.
    Returns list of (j, rhs_a, rhs_b, ps_a) with rhs cols into the [128,384]
    S tile (P window at 0, F window at 192) and psum column offset."""
    L, R = 512 * tt, 512 * tt + 512
    out = []
    for j in range(max(0, 4 * tt - 1), min(nR, 4 * tt + 5)):
        # P window: t in [128j, 128j+128+pad)
        a = max(L, 128 * j, 0)
        b = min(R, 128 * j + 128 + pad, T)
        if a < b:
            out.append((j, a - 128 * j, b - 128 * j, a - L))
        # F window: t in [128j-pad, 128j+128); rel cols based at 128j-64
        a = max(L, 128 * j - pad, 0)
        b = min(R, 128 * j + 128, T)
        if a < b:
            w0 = 128 * j - 64
            out.append((j, 192 + a - w0, 192 + b - w0, a - L))
    return out


def build_nc(T=T_FULL, num_devices=8, has_b1=False):
    nT = T // 512            # 512-wide time tiles
    nR = T // 128            # 128-wide token blocks

    nc = bacc.Bacc("TRN2", target_bir_lowering=False, debug=False,
                   num_devices=num_devices)
    # only lrelu(x) is shipped (host-precomputed, bf16): iteration-0's
    # xt tiles come straight off DMA, and the residual base is recovered
    # exactly on-chip via x = min(10*lrelu(x), lrelu(x)) -- no separate
    # x stream at all, halving input HBM traffic
    xth_d = nc.declare_dram_parameter("xth", [128, 2, T], BF16,
                                      isOutput=False)
    wpf_d = nc.declare_dram_parameter("wpf", [NITER, 128, 2, 512], BF16,
                                      isOutput=False)
    wcc_d = nc.declare_dram_parameter("wcc", [NITER, 128, 2, 2, 128], BF16,
                                      isOutput=False)
    wa_d = nc.declare_dram_parameter("wa", [NITER, 128, 3, 2, 2, 128], BF16,
                                     isOutput=False)
    b3_d = nc.declare_dram_parameter("b3", [NITER, 128, 2], F32,
                                     isOutput=False)
    rel_d = nc.declare_dram_parameter("rel", [NITER, 128, nR, 384],
                                      mybir.dt.int8, isOutput=False)
    iota_d = nc.declare_dram_parameter("iota", [128, 1], F32, isOutput=False)
    if has_b1:
        b1_d = nc.declare_dram_parameter("b1", [NITER, 128, 2], F32,
                                         isOutput=False)
    out_d = nc.declare_dram_parameter("out", [2, 128, T], F32, isOutput=True)

    with tile.TileContext(nc) as tc, ExitStack() as ctx:
        xpool = ctx.enter_context(tc.tile_pool(name="xres", bufs=1))
        stg = ctx.enter_context(tc.tile_pool(name="stg", bufs=4))
        tmpp = ctx.enter_context(tc.tile_pool(name="tmpp", bufs=2))
        stp = ctx.enter_context(tc.tile_pool(name="stp", bufs=13))
        sp = ctx.enter_context(tc.tile_pool(name="sp", bufs=13))
        relp = ctx.enter_context(tc.tile_pool(name="relp", bufs=2))
        xtp = ctx.enter_context(tc.tile_pool(name="xtp", bufs=16))
        vp = ctx.enter_context(tc.tile_pool(name="vp", bufs=4))
        wts = ctx.enter_context(tc.tile_pool(name="wts", bufs=2))
        cst = ctx.enter_context(tc.tile_pool(name="cst", bufs=1))
        ps_ps = ctx.enter_context(tc.tile_pool(name="ps", bufs=6, space="PSUM"))
        pc_ps = ctx.enter_context(tc.tile_pool(name="pc", bufs=2, space="PSUM"))
        pu_ps = pw_ps = ps_ps

        # all parameter DMAs dispatch from the sync engine: the scalar
        # (ACT) engine must stay dispatch-free, or flow-control waits on
        # DMA semaphore slots stall its activation stream
        def load_rel_blocks(rel_sb, i, b0, nblk):
            sl = bass.ds(b0, nblk)
            nc.sync.dma_start(rel_sb[:, sl, :], rel_d[i][:, sl, :])

        def load_w(i, what):
            # contiguous per-partition rows: fast, small DMAs
            if what == "wpf":
                # two dispatches: the first B matmul only needs half 0
                t = wts.tile([128, 2, 512], BF16, tag="wpf")
                nc.sync.dma_start(t[:, 0, :], wpf_d[i][:, 0, :])
                nc.sync.dma_start(t[:, 1, :], wpf_d[i][:, 1, :])
            elif what == "wcc":
                t = wts.tile([128, 2, 2, 128], BF16, tag="wcc")
                nc.sync.dma_start(t[:, :, :, :], wcc_d[i])
            elif what == "wa":
                t = wts.tile([128, 3, 2, 2, 128], BF16, tag="wa")
                nc.sync.dma_start(t[:, :, :, :, :], wa_d[i])
            elif what == "b3":
                t = wts.tile([128, 2], F32, tag="b3")
                nc.sync.dma_start(t[:, :], b3_d[i])
            elif what == "b1":
                t = wts.tile([128, 2], F32, tag="b1")
                nc.sync.dma_start(t[:, :], b1_d[i])
            return t

        # ---- startup ----
        # Everything dispatches from the sync engine, interleaved in
        # NEED-time order: the DMA semaphore-slot flow control then paces
        # later transfers behind earlier ones, which is exactly the
        # priority we want.  The residual accumulates in bf16 in x_sb
        # (seeded on the fly from inv-lrelu of the xt tiles); only the
        # final output stages through fp32 tiles for the DMA out.
        x_sb = xpool.tile([128, 2, T], BF16)

        xt_iter0 = []
        for t in range(nT):
            xh = xtp.tile([128, 2, 512], BF16, tag="xt")
            xt_iter0.append(xh)

        def load_xth(a, b, eng=None):
            t = a // 512
            ts0 = a - 512 * t
            (eng or nc.sync).dma_start(xt_iter0[t][:, :, ts0:ts0 + b - a],
                                       xth_d[:, :, a:b])

        w0 = {"wpf": load_w(0, "wpf")}
        load_xth(0, 128)
        load_xth(128, 512)
        iota_sb = cst.tile([128, 1], F32)
        nc.sync.dma_start(iota_sb[:, :], iota_d[:, :])
        rel0 = relp.tile([128, nR, 384], mybir.dt.int8, tag="rel")
        load_rel_blocks(rel0, 0, 0, min(8, nR))
        if T > 512:
            load_xth(512, 768)
            load_xth(768, min(1024, T))
        w0["wcc"] = load_w(0, "wcc")
        # tiles 2-3 dispatch from the scalar engine (idle until its first
        # st-copy): two queues drive more transfers through the DMA
        # latency pipe while the early feed rate is still ramping
        for t in range(2, min(4, nT)):
            load_xth(512 * t, 512 * t + 512, eng=nc.scalar)
        for t in range(4, min(6, nT)):
            load_xth(512 * t, 512 * t + 512)
        if nR > 8:
            load_rel_blocks(rel0, 0, 8, min(8, nR - 8))
        w0["wa"] = load_w(0, "wa")
        if nR > 16:
            load_rel_blocks(rel0, 0, 16, min(16, nR - 16))
        w0["b3"] = load_w(0, "b3")
        if has_b1:
            w0["b1"] = load_w(0, "b1")
        for t in range(6, min(9, nT)):
            load_xth(512 * t, 512 * t + 512)
        if nR > 32:
            load_rel_blocks(rel0, 0, 32, min(16, nR - 32))
        for t in range(9, min(12, nT)):
            load_xth(512 * t, 512 * t + 512)
        if nR > 48:
            load_rel_blocks(rel0, 0, 48, nR - 48)
        for t in range(12, nT):
            load_xth(512 * t, 512 * t + 512)

        st_tiles = [None] * nR
        S_tiles = [None] * nR
        xt_tiles = [None] * nT
        xt_next = [None] * nT
        v_tiles = [None] * nT

        def emit_A(tt):
            tsl = bass.ts(tt, 512)
            xt = xtp.tile([128, 2, 512], BF16, tag="xt")
            nc.vector.scalar_tensor_tensor(
                xt[:, :, :], x_sb[:, :, tsl], SLOPE, x_sb[:, :, tsl],
                OP.mult, OP.max)
            return xt

        def emit_conv3(tt, wa_sb, b3_sb, last, first=False):
            tsl = bass.ts(tt, 512)
            if first:
                # iteration 0 recovers the residual base exactly from
                # the (host-lrelu'd) xt tile: x = min(10*lrelu, lrelu)
                tmp = tmpp.tile([128, 2, 512], BF16, tag="tmp")
                nc.vector.scalar_tensor_tensor(
                    tmp[:, :, :], xt_tiles[tt][:, :, :], 10.0,
                    xt_tiles[tt][:, :, :], OP.mult, OP.min)
            def chain(py, ob, a, w):
                j = 0
                for k in range(3):
                    for cb in range(2):
                        nc.tensor.matmul(
                            py[:, a:a + w], wa_sb[:, k, cb, ob, :],
                            v_tiles[tt][:, cb, k + a:k + a + w],
                            start=(j == 0), stop=(j == 5))
                        j += 1

            def drain(py, ob, a, w):
                # final iteration: residual lands in fp32 staging tiles
                # (DMA cannot convert dtypes)
                hsl = bass.ds(512 * tt + a, w)
                sg = stg.tile([128, 512], F32, tag="stg")
                nc.vector.scalar_tensor_tensor(
                    sg[:, 0:w], py[:, a:a + w], b3_sb[:, ob:ob + 1],
                    x_sb[:, ob, hsl], OP.add, OP.add)
                nc.sync.dma_start(out_d[ob, :, hsl], sg[:, 0:w])

            if last and tt == nT - 1:
                # very last tile: quarter-chains interleaved across the
                # two output halves so the drain STTs + out-DMAs overlap
                # the remaining matmuls instead of gating them
                py0 = pc_ps.tile([128, 512], F32, tag="pc")
                py1 = pc_ps.tile([128, 512], F32, tag="pc")
                chain(py0, 0, 0, 256)
                chain(py1, 1, 0, 256)
                drain(py0, 0, 0, 256)
                chain(py0, 0, 256, 256)
                drain(py1, 1, 0, 256)
                chain(py1, 1, 256, 256)
                drain(py0, 0, 256, 256)
                drain(py1, 1, 256, 256)
            else:
                for ob in range(2):
                    py = pc_ps.tile([128, 512], F32, tag="pc")
                    chain(py, ob, 0, 512)
                    if last:
                        drain(py, ob, 0, 512)
                    elif first:
                        nc.vector.scalar_tensor_tensor(
                            x_sb[:, ob, tsl], py[:, :], b3_sb[:, ob:ob + 1],
                            tmp[:, ob, :], OP.add, OP.add)
                    else:
                        nc.vector.scalar_tensor_tensor(
                            x_sb[:, ob, tsl], py[:, :], b3_sb[:, ob:ob + 1],
                            x_sb[:, ob, tsl], OP.add, OP.add)

        cur_w = cur_rel = None
        nxt_w = nxt_rel = None
        for i in range(NITER):
            if i == 0:
                cur_w, cur_rel = w0, rel0
            else:
                cur_w, cur_rel = nxt_w, nxt_rel
            nxt_w, nxt_rel = {}, None
            wpf_sb, wcc_sb, wa_sb = cur_w["wpf"], cur_w["wcc"], cur_w["wa"]
            b3_sb, rel_sb = cur_w["b3"], cur_rel
            b1_sb = cur_w.get("b1")
            pad = PADS[i]

            if i != 0:
                xt_tiles, xt_next = xt_next, [None] * nT

            def emit_B(b):
                tt, off = b // 4, (b % 4) * 128
                ps = pu_ps.tile([128, 512], F32, tag="ps")
                nc.tensor.matmul(ps[:, :], xt_tiles[tt][:, 0, off:off + 128],
                                 wpf_sb[:, 0, :], start=True, stop=False)
                nc.tensor.matmul(ps[:, :], xt_tiles[tt][:, 1, off:off + 128],
                                 wpf_sb[:, 1, :], start=False, stop=True)
                st = stp.tile([128, 512], BF16, tag="st")
                st_tiles[b] = st
                nc.scalar.activation(st[:, :], ps[:, :], AF.Copy)

            def emit_S(b):
                S = sp.tile([128, 384], BF16, tag="S")
                S_tiles[b] = S
                nc.vector.tensor_scalar(S[:, :], rel_sb[:, b, :],
                                        iota_sb[:, 0:1], None, OP.is_equal)

            def emit_T(tt):
                pcs = _pieces(tt, nR, T, pad)
                v = vp.tile([128, 2, 516], BF16, tag="v")
                v_tiles[tt] = v
                for ob in range(2):
                    pw = pw_ps.tile([128, 512], F32, tag="ps")
                    nc.tensor.matmul(pw[:, :], wcc_sb[:, 0, ob, :],
                                     xt_tiles[tt][:, 0, :],
                                     start=True, stop=False)
                    for (j, ra, rb, pa) in pcs:
                        half = 0 if ra < 192 else 256
                        nc.tensor.matmul(
                            pw[:, pa:pa + rb - ra],
                            st_tiles[j][:, half + ob * 128:
                                        half + ob * 128 + 128],
                            S_tiles[j][:, ra:rb], start=False, stop=False)
                    nc.tensor.matmul(pw[:, :], wcc_sb[:, 1, ob, :],
                                     xt_tiles[tt][:, 1, :],
                                     start=False, stop=True)
                    if has_b1:
                        nc.scalar.activation(v[:, ob, 1:513], pw[:, :],
                                             AF.Prelu, alpha=SLOPE,
                                             bias=b1_sb[:, ob:ob + 1])
                    else:
                        nc.scalar.activation(v[:, ob, 1:513], pw[:, :],
                                             AF.Prelu, alpha=SLOPE)
                if tt == 0:
                    nc.vector.memset(v[:, :, 0:1], 0.0)
                else:
                    nc.vector.tensor_copy(v[:, :, 0:1],
                                          v_tiles[tt - 1][:, :, 512:513])
                    nc.vector.tensor_copy(v_tiles[tt - 1][:, :, 513:514],
                                          v[:, :, 1:2])
                    emit_conv3(tt - 1, wa_sb, b3_sb, i == NITER - 1,
                               i == 0)
                    if i + 1 < NITER:
                        xt_next[tt - 1] = emit_A(tt - 1)

            for b in range(nR + LAG):
                # consume (T) before produce (B): a B block waiting on the
                # x stream must not head-block a ready T tile on the PE
                if b >= LAG and (b - LAG) % 4 == 0:
                    emit_T((b - LAG) // 4)
                if b < nR:
                    if i == 0 and b == 0:
                        # iteration-0 xt tiles arrive host-precomputed
                        # straight off DMA
                        for t2 in range(nT):
                            xt_tiles[t2] = xt_iter0[t2]
                    emit_B(b)
                    # iteration 0 defers the first S-builds until the
                    # tile-0/1 lrelus are emitted, so a late rel chunk 0
                    # cannot head-block the B-stage warmup on the DVE
                    if i != 0 or b > 4:
                        emit_S(b)
                    elif b == 4:
                        for bb in range(5):
                            emit_S(bb)
                    if i + 1 < NITER:
                        # prefetch next iteration's weights + rel on the
                        # (by now idle) sync queue, once the x stream and
                        # iteration-0 rel chunks have fully dispatched
                        if b == 36:
                            nxt_w["wpf"] = load_w(i + 1, "wpf")
                        elif b == 37:
                            nxt_w["wcc"] = load_w(i + 1, "wcc")
                        elif b == 38:
                            nxt_w["wa"] = load_w(i + 1, "wa")
                        elif b == 39:
                            nxt_w["b3"] = load_w(i + 1, "b3")
                            if has_b1:
                                nxt_w["b1"] = load_w(i + 1, "b1")
                        elif b == 40:
                            nxt_rel = relp.tile([128, nR, 384],
                                                mybir.dt.int8, tag="rel")
                            load_rel_blocks(nxt_rel, i + 1, 0, nR)
            nc.vector.memset(v_tiles[nT - 1][:, :, 513:514], 0.0)
            emit_conv3(nT - 1, wa_sb, b3_sb, i == NITER - 1, i == 0)
            if i + 1 < NITER:
                xt_next[nT - 1] = emit_A(nT - 1)

    nc.compile()
    return nc


def _to_bf16(a):
    return np.asarray(a, dtype=np.float32).astype(ml_dtypes.bfloat16)


def prep_in_maps(x, d, WC, bC, WP, bP, WF, bF, WA, bA, T=T_FULL):
    """Build the 8 per-core input maps from the full-problem arrays.
    Returns (in_maps, has_b1)."""
    x = np.asarray(x, dtype=np.float32)
    d = np.asarray(d, dtype=np.float32)
    WC, WP, WF, WA = (np.asarray(w, dtype=np.float32) for w in (WC, WP, WF, WA))
    bC, bP, bF, bA = (np.asarray(b, dtype=np.float32) for b in (bC, bP, bF, bA))
    nb = x.shape[0]
    nR = T // 128

    # weights stored in DRAM in the exact SBUF layout (partition dim first)
    wpf = np.empty((NITER, 128, 2, 512), np.float32)
    wcc = np.empty((NITER, 128, 2, 2, 128), np.float32)
    wa = np.empty((NITER, 128, 3, 2, 2, 128), np.float32)
    for i in range(NITER):
        wpfT = np.concatenate([WP[i].T, WF[i].T], axis=1)  # [c', 512]
        wpf[i] = wpfT.reshape(2, 128, 512).transpose(1, 0, 2)
        for cb in range(2):
            for ob in range(2):
                wcc[i, :, cb, ob] = \
                    WC[i][ob * 128:(ob + 1) * 128,
                          cb * 128:(cb + 1) * 128].T
        for k in range(3):
            waT = WA[i, :, :, k].T                         # [c', o]
            wa[i, :, k] = waT.reshape(2, 128, 2, 128) \
                .transpose(1, 0, 2, 3)
    b1 = (bC + bP + bF).astype(np.float32)                  # [NITER, 256]
    has_b1 = bool(np.any(b1 != 0))
    b3 = bA.reshape(NITER, 2, 128).transpose(0, 2, 1).copy()

    wpf, wcc, wa = _to_bf16(wpf), _to_bf16(wcc), _to_bf16(wa)
    iota = np.arange(128, dtype=np.float32).reshape(128, 1)

    tf = np.arange(T, dtype=np.float32)
    in_maps = []
    for b in range(nb):
        dv = d[b, 0].astype(np.float32)
        rel = np.full((NITER, nR, 384), -128, np.int8)
        for i, dil in enumerate(DILATIONS):
            dd = dv * np.float32(dil)
            idxP = np.round(tf - dd).astype(np.int64)
            idxF = np.round(tf + dd).astype(np.int64)
            for j in range(nR):
                # P window: t in [128j, 128j+192)
                a, e = 128 * j, min(128 * j + 192, T)
                hit = idxP[a:e] // 128 == j
                rel[i, j, 0:e - a] = np.where(
                    hit, idxP[a:e] - 128 * j, -128).astype(np.int8)
                # F window: t in [128j-64, 128j+128)
                w0 = 128 * j - 64
                a, e = max(0, w0), min(128 * j + 128, T)
                hit = idxF[a:e] // 128 == j
                rel[i, j, 192 + a - w0:192 + e - w0] = np.where(
                    hit, idxF[a:e] - 128 * j, -128).astype(np.int8)
        xf = _to_bf16(x[b].reshape(2, 128, T)).astype(np.float32)
        xth = _to_bf16(np.maximum(np.float32(0.1) * xf, xf)
                       .transpose(1, 0, 2))
        m = {
            "xth": xth,
            "wpf": wpf, "wcc": wcc, "wa": wa, "b3": b3,
            "rel": np.broadcast_to(rel[:, None], (NITER, 128, nR, 384)).copy(),
            "iota": iota,
        }
        if has_b1:
            m["b1"] = b1.reshape(NITER, 2, 128).transpose(0, 2, 1).copy()
        in_maps.append(m)
    return in_maps, has_b1


_nc_cache = {}


def kernel(**inputs) -> np.ndarray:
    T = inputs["x"].shape[2]
    in_maps, has_b1 = prep_in_maps(**inputs, T=T)
    key = (T, has_b1)
    if key not in _nc_cache:
        _nc_cache[key] = build_nc(T, has_b1=has_b1)
    nc = _nc_cache[key]
    res = run_bass_kernel_spmd(nc, in_maps, core_ids=list(range(8)))
    out = np.stack([np.asarray(res.results[i]["out"], dtype=np.float32)
                    .reshape(C, T) for i in range(8)])
    return out


# revision 82
# speedup vs baseline: 1.0065x; 1.0065x over previous
"""Trainium2 Bass kernel for nn_AdaptiveResBlock (8-core data-parallel).

Reference computation (per batch element b, C=256 channels, T=8192 time):
  for i, dil in enumerate((1, 2, 4)):
      xt = lrelu(x)
      xP, xF = time-gather of xt at round(t -/+ d*dil), zero out-of-range
      xt = WC@xt + WP@xP + WF@xF + biases        (1x1 convs over channels)
      xt = lrelu(xt)
      xt = conv3(xt, WA) + bias
      x = xt + x

Structure used:
  * The time-gather commutes with the 1x1 convs:
    WP @ gather(xt) == gather(WP @ xt).
  * B-stage: u^T = [WP;WF] @ xt computed token-major on the PE
    (xt-stationary matmuls), ACT-copied as bf16 st tiles [128 tok, 512].
  * Channel-major windowed one-hot gather: offsets are bounded by
    16*dilation, so every output token whose P-source lies in token
    block j sits in the (128+pad)-token window [128j, 128j+128+pad)
    with pad = 16*dil (F: [128j-pad, 128j+128)).  One matmul per
    (dir, c-half, src block):
      psum[c, t] += st[j][:, c-slice]^T @ S_dir[j][src, t-window]
    lands the gathered conv result DIRECTLY channel-major in PSUM, on top
    of the WC matmuls accumulating in the same bank -- no PE transposes,
    no separate gather pass.  S is built on DVE via is_equal of
    preloaded window-relative indices (int8, -128 sentinel) against an
    iota column; out-of-range indices match no block => free zero mask.
  * Gather windows are dilation-aware: iteration i only spans
    128 + 16*dil source tokens per block, cutting one-hot matmul
    columns by ~14% versus a fixed 192-token window.
  * v = Prelu(psum) straight from PSUM (ACT); conv3 runs channel-major
    off v tiles with 1-column halos; residual update and the next
    iteration's lrelu are fused right behind each tile (DVE).
  * Only lrelu(x) is shipped (host-precomputed bf16): iteration-0's xt
    tiles come straight off DMA with no on-chip lrelu, and the residual
    base is recovered exactly via x = min(10*lrelu(x), lrelu(x)) fused
    into iteration-0's conv3 drain.  The residual accumulates in bf16
    (every matmul consumer is bf16 anyway); the final iteration drains
    through fp32 staging tiles for the output DMA.
  * All DMAs dispatch from the sync engine in need-time order (the
    DMA-semaphore flow control then paces transfers by priority);
    weights are stored in DRAM pre-transposed to the exact SBUF layout
    so every weight DMA is a contiguous per-partition row; iteration
    i+1's weights/rel prefetch mid-iteration i.

Sharded data-parallel over B=8 across the 8 NeuronCores; weights
replicated; per-core window-index tensors precomputed from d on host.
"""

import numpy as np
import ml_dtypes
from contextlib import ExitStack

import concourse.bass as bass
import concourse.tile as tile
from concourse import mybir, bacc
from concourse.bass_utils import run_bass_kernel_spmd

F32 = mybir.dt.float32
BF16 = mybir.dt.bfloat16
AF = mybir.ActivationFunctionType
OP = mybir.AluOpType

B, C, T_FULL = 8, 256, 8192
DILATIONS = (1, 2, 4)
PADS = tuple(16 * d for d in DILATIONS)   # max gather offset per iteration
NITER = len(DILATIONS)
SLOPE = 0.1
LAG = 8        # B-stage leads the consume stage by this many token blocks


def _pieces(tt, nR, T, pad):
    """Gather matmul pieces for 512-token tile tt at max offset `pad`.
    Returns list of (j, rhs_a, rhs_b, ps_a) with rhs cols into the [128,384]
    S tile (P window at 0, F window at 192) and psum column offset."""
    L, R = 512 * tt, 512 * tt + 512
    out = []
    for j in range(max(0, 4 * tt - 1), min(nR, 4 * tt + 5)):
        # P window: t in [128j, 128j+128+pad)
        a = max(L, 128 * j, 0)
        b = min(R, 128 * j + 128 + pad, T)
        if a < b:
            out.append((j, a - 128 * j, b - 128 * j, a - L))
        # F window: t in [128j-pad, 128j+128); rel cols based at 128j-64
        a = max(L, 128 * j - pad, 0)
        b = min(R, 128 * j + 128, T)
        if a < b:
            w0 = 128 * j - 64
            out.append((j, 192 + a - w0, 192 + b - w0, a - L))
    return out


def build_nc(T=T_FULL, num_devices=8, has_b1=False):
    nT = T // 512            # 512-wide time tiles
    nR = T // 128            # 128-wide token blocks

    nc = bacc.Bacc("TRN2", target_bir_lowering=False, debug=False,
                   num_devices=num_devices)
    # only lrelu(x) is shipped (host-precomputed, bf16): iteration-0's
    # xt tiles come straight off DMA, and the residual base is recovered
    # exactly on-chip via x = min(10*lrelu(x), lrelu(x)) -- no separate
    # x stream at all, halving input HBM traffic
    xth_d = nc.declare_dram_parameter("xth", [128, 2, T], BF16,
                                      isOutput=False)
    wpf_d = nc.declare_dram_parameter("wpf", [NITER, 128, 2, 512], BF16,
                                      isOutput=False)
    wcc_d = nc.declare_dram_parameter("wcc", [NITER, 128, 2, 2, 128], BF16,
                                      isOutput=False)
    wa_d = nc.declare_dram_parameter("wa", [NITER, 128, 3, 2, 2, 128], BF16,
                                     isOutput=False)
    b3_d = nc.declare_dram_parameter("b3", [NITER, 128, 2], F32,
                                     isOutput=False)
    rel_d = nc.declare_dram_parameter("rel", [NITER, 128, nR, 384],
                                      mybir.dt.int8, isOutput=False)
    iota_d = nc.declare_dram_parameter("iota", [128, 1], F32, isOutput=False)
    if has_b1:
        b1_d = nc.declare_dram_parameter("b1", [NITER, 128, 2], F32,
                                         isOutput=False)
    out_d = nc.declare_dram_parameter("out", [2, 128, T], F32, isOutput=True)

    with tile.TileContext(nc) as tc, ExitStack() as ctx:
        xpool = ctx.enter_context(tc.tile_pool(name="xres", bufs=1))
        stg = ctx.enter_context(tc.tile_pool(name="stg", bufs=4))
        tmpp = ctx.enter_context(tc.tile_pool(name="tmpp", bufs=2))
        stp = ctx.enter_context(tc.tile_pool(name="stp", bufs=13))
        sp = ctx.enter_context(tc.tile_pool(name="sp", bufs=13))
        relp = ctx.enter_context(tc.tile_pool(name="relp", bufs=2))
        xtp = ctx.enter_context(tc.tile_pool(name="xtp", bufs=16))
        vp = ctx.enter_context(tc.tile_pool(name="vp", bufs=4))
        wts = ctx.enter_context(tc.tile_pool(name="wts", bufs=2))
        cst = ctx.enter_context(tc.tile_pool(name="cst", bufs=1))
        ps_ps = ctx.enter_context(tc.tile_pool(name="ps", bufs=6, space="PSUM"))
        pc_ps = ctx.enter_context(tc.tile_pool(name="pc", bufs=2, space="PSUM"))
        pu_ps = pw_ps = ps_ps

        # all parameter DMAs dispatch from the sync engine: the scalar
        # (ACT) engine must stay dispatch-free, or flow-control waits on
        # DMA semaphore slots stall its activation stream
        def load_rel_blocks(rel_sb, i, b0, nblk):
            sl = bass.ds(b0, nblk)
            nc.sync.dma_start(rel_sb[:, sl, :], rel_d[i][:, sl, :])

        def load_w(i, what):
            # contiguous per-partition rows: fast, small DMAs
            if what == "wpf":
                # two dispatches: the first B matmul only needs half 0
                t = wts.tile([128, 2, 512], BF16, tag="wpf")
                nc.sync.dma_start(t[:, 0, :], wpf_d[i][:, 0, :])
                nc.sync.dma_start(t[:, 1, :], wpf_d[i][:, 1, :])
            elif what == "wcc":
                t = wts.tile([128, 2, 2, 128], BF16, tag="wcc")
                nc.sync.dma_start(t[:, :, :, :], wcc_d[i])
            elif what == "wa":
                t = wts.tile([128, 3, 2, 2, 128], BF16, tag="wa")
                nc.sync.dma_start(t[:, :, :, :, :], wa_d[i])
            elif what == "b3":
                t = wts.tile([128, 2], F32, tag="b3")
                nc.sync.dma_start(t[:, :], b3_d[i])
            elif what == "b1":
                t = wts.tile([128, 2], F32, tag="b1")
                nc.sync.dma_start(t[:, :], b1_d[i])
            return t

        # ---- startup ----
        # Everything dispatches from the sync engine, interleaved in
        # NEED-time order: the DMA semaphore-slot flow control then paces
        # later transfers behind earlier ones, which is exactly the
        # priority we want.  The residual accumulates in bf16 in x_sb
        # (seeded on the fly from inv-lrelu of the xt tiles); only the
        # final output stages through fp32 tiles for the DMA out.
        x_sb = xpool.tile([128, 2, T], BF16)

        xt_iter0 = []
        for t in range(nT):
            xh = xtp.tile([128, 2, 512], BF16, tag="xt")
            xt_iter0.append(xh)

        def load_xth(a, b, eng=None):
            t = a // 512
            ts0 = a - 512 * t
            (eng or nc.sync).dma_start(xt_iter0[t][:, :, ts0:ts0 + b - a],
                                       xth_d[:, :, a:b])

        w0 = {"wpf": load_w(0, "wpf")}
        load_xth(0, 128)
        load_xth(128, 512)
        iota_sb = cst.tile([128, 1], F32)
        nc.sync.dma_start(iota_sb[:, :], iota_d[:, :])
        rel0 = relp.tile([128, nR, 384], mybir.dt.int8, tag="rel")
        load_rel_blocks(rel0, 0, 0, min(8, nR))
        if T > 512:
            load_xth(512, 768)
            load_xth(768, min(1024, T))
        w0["wcc"] = load_w(0, "wcc")
        for t in range(2, min(4, nT)):
            load_xth(512 * t, 512 * t + 512)
        if nR > 8:
            load_rel_blocks(rel0, 0, 8, min(8, nR - 8))
        w0["wa"] = load_w(0, "wa")
        for t in range(4, min(6, nT)):
            load_xth(512 * t, 512 * t + 512)
        if nR > 16:
            load_rel_blocks(rel0, 0, 16, min(16, nR - 16))
        w0["b3"] = load_w(0, "b3")
        if has_b1:
            w0["b1"] = load_w(0, "b1")
        for t in range(6, min(9, nT)):
            load_xth(512 * t, 512 * t + 512)
        if nR > 32:
            load_rel_blocks(rel0, 0, 32, min(16, nR - 32))
        for t in range(9, min(12, nT)):
            load_xth(512 * t, 512 * t + 512)
        if nR > 48:
            load_rel_blocks(rel0, 0, 48, nR - 48)
        for t in range(12, nT):
            load_xth(512 * t, 512 * t + 512)

        st_tiles = [None] * nR
        S_tiles = [None] * nR
        xt_tiles = [None] * nT
        xt_next = [None] * nT
        v_tiles = [None] * nT

        def emit_A(tt):
            tsl = bass.ts(tt, 512)
            xt = xtp.tile([128, 2, 512], BF16, tag="xt")
            nc.vector.scalar_tensor_tensor(
                xt[:, :, :], x_sb[:, :, tsl], SLOPE, x_sb[:, :, tsl],
                OP.mult, OP.max)
            return xt

        def emit_conv3(tt, wa_sb, b3_sb, last, first=False):
            tsl = bass.ts(tt, 512)
            if first:
                # iteration 0 recovers the residual base exactly from
                # the (host-lrelu'd) xt tile: x = min(10*lrelu, lrelu)
                tmp = tmpp.tile([128, 2, 512], BF16, tag="tmp")
                nc.vector.scalar_tensor_tensor(
                    tmp[:, :, :], xt_tiles[tt][:, :, :], 10.0,
                    xt_tiles[tt][:, :, :], OP.mult, OP.min)
            def chain(py, ob, a, w):
                j = 0
                for k in range(3):
                    for cb in range(2):
                        nc.tensor.matmul(
                            py[:, a:a + w], wa_sb[:, k, cb, ob, :],
                            v_tiles[tt][:, cb, k + a:k + a + w],
                            start=(j == 0), stop=(j == 5))
                        j += 1

            def drain(py, ob, a, w):
                # final iteration: residual lands in fp32 staging tiles
                # (DMA cannot convert dtypes)
                hsl = bass.ds(512 * tt + a, w)
                sg = stg.tile([128, 512], F32, tag="stg")
                nc.vector.scalar_tensor_tensor(
                    sg[:, 0:w], py[:, a:a + w], b3_sb[:, ob:ob + 1],
                    x_sb[:, ob, hsl], OP.add, OP.add)
                nc.sync.dma_start(out_d[ob, :, hsl], sg[:, 0:w])

            if last and tt == nT - 1:
                # very last tile: quarter-chains interleaved across the
                # two output halves so the drain STTs + out-DMAs overlap
                # the remaining matmuls instead of gating them
                py0 = pc_ps.tile([128, 512], F32, tag="pc")
                py1 = pc_ps.tile([128, 512], F32, tag="pc")
                chain(py0, 0, 0, 256)
                chain(py1, 1, 0, 256)
                drain(py0, 0, 0, 256)
                chain(py0, 0, 256, 256)
                drain(py1, 1, 0, 256)
                chain(py1, 1, 256, 256)
                drain(py0, 0, 256, 256)
                drain(py1, 1, 256, 256)
            else:
                for ob in range(2):
                    py = pc_ps.tile([128, 512], F32, tag="pc")
                    chain(py, ob, 0, 512)
                    if last:
                        drain(py, ob, 0, 512)
                    elif first:
                        nc.vector.scalar_tensor_tensor(
                            x_sb[:, ob, tsl], py[:, :], b3_sb[:, ob:ob + 1],
                            tmp[:, ob, :], OP.add, OP.add)
                    else:
                        nc.vector.scalar_tensor_tensor(
                            x_sb[:, ob, tsl], py[:, :], b3_sb[:, ob:ob + 1],
                            x_sb[:, ob, tsl], OP.add, OP.add)

        cur_w = cur_rel = None
        nxt_w = nxt_rel = None
        for i in range(NITER):
            if i == 0:
                cur_w, cur_rel = w0, rel0
            else:
                cur_w, cur_rel = nxt_w, nxt_rel
            nxt_w, nxt_rel = {}, None
            wpf_sb, wcc_sb, wa_sb = cur_w["wpf"], cur_w["wcc"], cur_w["wa"]
            b3_sb, rel_sb = cur_w["b3"], cur_rel
            b1_sb = cur_w.get("b1")
            pad = PADS[i]

            if i != 0:
                xt_tiles, xt_next = xt_next, [None] * nT

            def emit_B(b):
                tt, off = b // 4, (b % 4) * 128
                ps = pu_ps.tile([128, 512], F32, tag="ps")
                nc.tensor.matmul(ps[:, :], xt_tiles[tt][:, 0, off:off + 128],
                                 wpf_sb[:, 0, :], start=True, stop=False)
                nc.tensor.matmul(ps[:, :], xt_tiles[tt][:, 1, off:off + 128],
                                 wpf_sb[:, 1, :], start=False, stop=True)
                st = stp.tile([128, 512], BF16, tag="st")
                st_tiles[b] = st
                nc.scalar.activation(st[:, :], ps[:, :], AF.Copy)

            def emit_S(b):
                S = sp.tile([128, 384], BF16, tag="S")
                S_tiles[b] = S
                nc.vector.tensor_scalar(S[:, :], rel_sb[:, b, :],
                                        iota_sb[:, 0:1], None, OP.is_equal)

            def emit_T(tt):
                pcs = _pieces(tt, nR, T, pad)
                v = vp.tile([128, 2, 516], BF16, tag="v")
                v_tiles[tt] = v
                for ob in range(2):
                    pw = pw_ps.tile([128, 512], F32, tag="ps")
                    nc.tensor.matmul(pw[:, :], wcc_sb[:, 0, ob, :],
                                     xt_tiles[tt][:, 0, :],
                                     start=True, stop=False)
                    for (j, ra, rb, pa) in pcs:
                        half = 0 if ra < 192 else 256
                        nc.tensor.matmul(
                            pw[:, pa:pa + rb - ra],
                            st_tiles[j][:, half + ob * 128:
                                        half + ob * 128 + 128],
                            S_tiles[j][:, ra:rb], start=False, stop=False)
                    nc.tensor.matmul(pw[:, :], wcc_sb[:, 1, ob, :],
                                     xt_tiles[tt][:, 1, :],
                                     start=False, stop=True)
                    if has_b1:
                        nc.scalar.activation(v[:, ob, 1:513], pw[:, :],
                                             AF.Prelu, alpha=SLOPE,
                                             bias=b1_sb[:, ob:ob + 1])
                    else:
                        nc.scalar.activation(v[:, ob, 1:513], pw[:, :],
                                             AF.Prelu, alpha=SLOPE)
                if tt == 0:
                    nc.vector.memset(v[:, :, 0:1], 0.0)
                else:
                    nc.vector.tensor_copy(v[:, :, 0:1],
                                          v_tiles[tt - 1][:, :, 512:513])
                    nc.vector.tensor_copy(v_tiles[tt - 1][:, :, 513:514],
                                          v[:, :, 1:2])
                    emit_conv3(tt - 1, wa_sb, b3_sb, i == NITER - 1,
                               i == 0)
                    if i + 1 < NITER:
                        xt_next[tt - 1] = emit_A(tt - 1)

            for b in range(nR + LAG):
                # consume (T) before produce (B): a B block waiting on the
                # x stream must not head-block a ready T tile on the PE
                if b >= LAG and (b - LAG) % 4 == 0:
                    emit_T((b - LAG) // 4)
                if b < nR:
                    if i == 0 and b == 0:
                        # iteration-0 xt tiles arrive host-precomputed
                        # straight off DMA
                        for t2 in range(nT):
                            xt_tiles[t2] = xt_iter0[t2]
                    emit_B(b)
                    # iteration 0 defers the first S-builds until the
                    # tile-0/1 lrelus are emitted, so a late rel chunk 0
                    # cannot head-block the B-stage warmup on the DVE
                    if i != 0 or b > 4:
                        emit_S(b)
                    elif b == 4:
                        for bb in range(5):
                            emit_S(bb)
                    if i + 1 < NITER:
                        # prefetch next iteration's weights + rel on the
                        # (by now idle) sync queue, once the x stream and
                        # iteration-0 rel chunks have fully dispatched
                        if b == 36:
                            nxt_w["wpf"] = load_w(i + 1, "wpf")
                        elif b == 37:
                            nxt_w["wcc"] = load_w(i + 1, "wcc")
                        elif b == 38:
                            nxt_w["wa"] = load_w(i + 1, "wa")
                        elif b == 39:
                            nxt_w["b3"] = load_w(i + 1, "b3")
                            if has_b1:
                                nxt_w["b1"] = load_w(i + 1, "b1")
                        elif b == 40:
                            nxt_rel = relp.tile([128, nR, 384],
                                                mybir.dt.int8, tag="rel")
                            load_rel_blocks(nxt_rel, i + 1, 0, nR)
            nc.vector.memset(v_tiles[nT - 1][:, :, 513:514], 0.0)
            emit_conv3(nT - 1, wa_sb, b3_sb, i == NITER - 1, i == 0)
            if i + 1 < NITER:
                xt_next[nT - 1] = emit_A(nT - 1)

    nc.compile()
    return nc


def _to_bf16(a):
    return np.asarray(a, dtype=np.float32).astype(ml_dtypes.bfloat16)


def prep_in_maps(x, d, WC, bC, WP, bP, WF, bF, WA, bA, T=T_FULL):
    """Build the 8 per-core input maps from the full-problem arrays.
    Returns (in_maps, has_b1)."""
    x = np.asarray(x, dtype=np.float32)
    d = np.asarray(d, dtype=np.float32)
    WC, WP, WF, WA = (np.asarray(w, dtype=np.float32) for w in (WC, WP, WF, WA))
    bC, bP, bF, bA = (np.asarray(b, dtype=np.float32) for b in (bC, bP, bF, bA))
    nb = x.shape[0]
    nR = T // 128

    # weights stored in DRAM in the exact SBUF layout (partition dim first)
    wpf = np.empty((NITER, 128, 2, 512), np.float32)
    wcc = np.empty((NITER, 128, 2, 2, 128), np.float32)
    wa = np.empty((NITER, 128, 3, 2, 2, 128), np.float32)
    for i in range(NITER):
        wpfT = np.concatenate([WP[i].T, WF[i].T], axis=1)  # [c', 512]
        wpf[i] = wpfT.reshape(2, 128, 512).transpose(1, 0, 2)
        for cb in range(2):
            for ob in range(2):
                wcc[i, :, cb, ob] = \
                    WC[i][ob * 128:(ob + 1) * 128,
                          cb * 128:(cb + 1) * 128].T
        for k in range(3):
            waT = WA[i, :, :, k].T                         # [c', o]
            wa[i, :, k] = waT.reshape(2, 128, 2, 128) \
                .transpose(1, 0, 2, 3)
    b1 = (bC + bP + bF).astype(np.float32)                  # [NITER, 256]
    has_b1 = bool(np.any(b1 != 0))
    b3 = bA.reshape(NITER, 2, 128).transpose(0, 2, 1).copy()

    wpf, wcc, wa = _to_bf16(wpf), _to_bf16(wcc), _to_bf16(wa)
    iota = np.arange(128, dtype=np.float32).reshape(128, 1)

    tf = np.arange(T, dtype=np.float32)
    in_maps = []
    for b in range(nb):
        dv = d[b, 0].astype(np.float32)
        rel = np.full((NITER, nR, 384), -128, np.int8)
        for i, dil in enumerate(DILATIONS):
            dd = dv * np.float32(dil)
            idxP = np.round(tf - dd).astype(np.int64)
            idxF = np.round(tf + dd).astype(np.int64)
            for j in range(nR):
                # P window: t in [128j, 128j+192)
                a, e = 128 * j, min(128 * j + 192, T)
                hit = idxP[a:e] // 128 == j
                rel[i, j, 0:e - a] = np.where(
                    hit, idxP[a:e] - 128 * j, -128).astype(np.int8)
                # F window: t in [128j-64, 128j+128)
                w0 = 128 * j - 64
                a, e = max(0, w0), min(128 * j + 128, T)
                hit = idxF[a:e] // 128 == j
                rel[i, j, 192 + a - w0:192 + e - w0] = np.where(
                    hit, idxF[a:e] - 128 * j, -128).astype(np.int8)
        xf = _to_bf16(x[b].reshape(2, 128, T)).astype(np.float32)
        xth = _to_bf16(np.maximum(np.float32(0.1) * xf, xf)
                       .transpose(1, 0, 2))
        m = {
            "xth": xth,
            "wpf": wpf, "wcc": wcc, "wa": wa, "b3": b3,
            "rel": np.broadcast_to(rel[:, None], (NITER, 128, nR, 384)).copy(),
            "iota": iota,
        }
        if has_b1:
            m["b1"] = b1.reshape(NITER, 2, 128).transpose(0, 2, 1).copy()
        in_maps.append(m)
    return in_maps, has_b1


_nc_cache = {}


def kernel(**inputs) -> np.ndarray:
    T = inputs["x"].shape[2]
    in_maps, has_b1 = prep_in_maps(**inputs, T=T)
    key = (T, has_b1)
    if key not in _nc_cache:
        _nc_cache[key] = build_nc(T, has_b1=has_b1)
    nc = _nc_cache[key]
    res = run_bass_kernel_spmd(nc, in_maps, core_ids=list(range(8)))
    out = np.stack([np.asarray(res.results[i]["out"], dtype=np.float32)
                    .reshape(C, T) for i in range(8)])
    return out
